# revision 1
# baseline (speedup 1.0000x reference)
"""LocalVarianceNet Trainium2 kernel.

Computes E[x^2] - E[x]^2 over a 7x7 circular (wrap-padded) window, per
channel, for x of shape [16, 3, 512, 512] fp32.

Strategy (data parallel over 8 cores, 6 planes of 512x512 per core):
  Both separable box-filter passes run on the Tensor engine as banded
  matmuls. matmul(out, lhsT=data_chunk, rhs=B_band) computes
  data_chunk^T @ B_band: it filters the partition dim of the data while
  transposing it, so two passes compose back to natural orientation:
      pass1: Yt = X^T  B   (vertical sum over rows, output transposed)
      pass2: Z  = Yt^T B   (horizontal sum over cols, natural output)

  PSUM free-dim coordinates are rotated by +3 (c = i + 3 mod 512), which
  makes every 128-row chunk's band contribution a contiguous column
  range of ONE shared triangular band matrix Bband[kl, c] = 1 iff
  kl <= c <= kl+6 ([128, 134] incl. both wrap corners). 5 matmuls per
  output bank. Intermediates are copied PSUM->SBUF with the +3 rotation
  undone in the copy (split at col 3, same total cost), so pass-2
  stationary slices start at ic*128 — 4-byte aligned, keeping
  LDWEIGHTS on the fast path, with no wrap halo; the final +3 rotation
  of pass 2 is undone by the output DMA (a 509-col and a 3-col piece).

  Data is cast to fp16 on the inbound DMA (weight loads of the data
  chunks dominate Tensor-engine time; fp16 enables fast weight load),
  all matmul accumulation stays fp32 in PSUM, and the final variance is
  written back as fp16 (rounding ~5e-4 relative, well inside the 2e-2
  budget) to halve outbound HBM traffic; the host casts back to fp32.
"""

import numpy as np

P = 128
HW = 512
PAD = 3  # window 7 -> halo 3
NCH = 4  # 512 / 128 chunks
BW = P + 2 * PAD  # 134: band tile width
N_CORES = 8
PLANES_PER_CORE = 6  # (16 images * 3 channels) / 8 cores


def _make_bmat(np_dtype):
    """Triangular band tile [128, 134]: B[kl, c] = 1 iff kl <= c <= kl+6."""
    kl = np.arange(P)[:, None]
    c = np.arange(BW)[None, :]
    return np.ascontiguousarray(((kl <= c) & (c <= kl + 2 * PAD)).astype(np_dtype))


def _band_pass(nc, ps, lhsT_of, bm, sim_safe):
    """Circular 7-band filter into psum ps [128, 512] (rotated coords).

    ps[m, c] = sum_k lhsT_of(chunk(k))[kl, m] * B[k, (c - 3) mod 512]

    Chunk kc writes psum cols [128*kc, 128*kc + 134) (mod 512, the kc=3
    tail wraps to [0, 6)), always with rhs = the shared triangular band
    tile. sim_safe additionally splits the 6-col overlaps so every
    matmul's PSUM region is uniformly first-write or accumulate
    (CoreSim models has_written at instruction granularity).
    """
    OV = 2 * PAD  # 6-col overlap between adjacent chunk bands
    seq = []
    if sim_safe:
        seq.append((0, bm[:, 0:BW], ps[:, 0:BW], True))
        for kc in range(1, NCH):
            lo = kc * P
            w = BW if kc < NCH - 1 else P
            seq.append((kc, bm[:, 0:OV], ps[:, lo : lo + OV], False))
            seq.append((kc, bm[:, OV:w], ps[:, lo + OV : lo + w], False))
        seq.append((NCH - 1, bm[:, P:BW], ps[:, 0:OV], False))
    else:
        seq.append((0, bm[:, 0:BW], ps[:, 0:BW], True))
        for kc in range(1, NCH - 1):
            lo = kc * P
            seq.append((kc, bm[:, 0:BW], ps[:, lo : lo + BW], False))
        seq.append((NCH - 1, bm[:, 0:P], ps[:, (NCH - 1) * P : HW], False))
        seq.append((NCH - 1, bm[:, P:BW], ps[:, 0:OV], False))
    n = len(seq)
    for i, (kc, rh, out, start) in enumerate(seq):
        nc.tensor.matmul(out, lhsT_of(kc), rh, start=start, stop=(i == n - 1))


def build(n_planes=PLANES_PER_CORE, sim_safe=False):
    import concourse.mybir as mybir
    from concourse import bacc
    from concourse.tile import TileContext

    f16 = mybir.dt.float16
    f32 = mybir.dt.float32
    SQ = mybir.ActivationFunctionType.Square
    MUL = mybir.AluOpType.mult
    SUB = mybir.AluOpType.subtract
    INV = 1.0 / 49.0
    HB = HW + PAD  # 515: halo-extended width of the Yt tiles

    nc = bacc.Bacc("TRN2", target_bir_lowering=False)
    x_d = nc.declare_dram_parameter("x", [n_planes, HW, HW], f32, isOutput=False)
    b_d = nc.declare_dram_parameter("bmat", [P, BW], f16, isOutput=False)
    o_d = nc.declare_dram_parameter("out", [n_planes, HW, HW], f16, isOutput=True)

    with TileContext(nc) as tc:
        with (
            tc.tile_pool(name="const", bufs=1) as constp,
            tc.tile_pool(name="xin", bufs=4) as xinp,
            tc.tile_pool(name="xsq", bufs=3) as xsqp,
            tc.tile_pool(name="yt", bufs=3) as ytp,
            tc.tile_pool(name="tsq", bufs=3) as tsqp,
            tc.tile_pool(name="outp", bufs=3) as outpp,
            tc.tile_pool(name="psA", bufs=2, space="PSUM") as psAp,
            tc.tile_pool(name="psZ", bufs=1, space="PSUM") as psZp,
        ):
            bm_t = constp.tile([P, BW], f16)
            nc.sync.dma_start(out=bm_t[:], in_=b_d[:, :])
            bm = bm_t[:]

            # ~5us of dense junk matmuls (long N, high array duty) trip the
            # PE clock-gate to full rate during the first input DMA.
            junk = constp.tile([P, HW], f16)
            nc.vector.memset(junk[:], 0.0)
            warm = psAp.tile([P, 2 * HW], f32, tag="ps")
            for w in range(12):
                nc.tensor.matmul(
                    warm[:, 0:HW], bm[:, 0:P], junk[:],
                    start=(w == 0), stop=(w == 11),
                )

            for p in range(n_planes):
                xin = xinp.tile([P, NCH, HW], f16)
                src = x_d[p].rearrange("(kc q) c -> q kc c", q=P)
                if p == 0:
                    # split plane 0's load by chunk-halves: the band passes'
                    # first matmuls only need chunk kc=0, so compute starts
                    # as soon as the first half lands
                    nc.gpsimd.dma_start(out=xin[:, 0:2, :], in_=src[:, 0:2, :])
                    nc.gpsimd.dma_start(out=xin[:, 2:4, :], in_=src[:, 2:4, :])
                else:
                    nc.gpsimd.dma_start(out=xin[:], in_=src)
                # square in two column halves: pass-1 x^2 jc-pair 0 only needs
                # cols [0,256), so it starts before the full square finishes
                xsq = xsqp.tile([P, NCH, HW], f16)
                for h in range(2):
                    sl = slice(h * HW // 2, (h + 1) * HW // 2)
                    nc.gpsimd.tensor_mul(
                        out=xsq[:, :, sl], in0=xin[:, :, sl], in1=xin[:, :, sl]
                    )

                yts = {}
                for t, src in (("x", xin), ("x2", xsq)):
                    yt = ytp.tile([P, NCH, HW], f16, tag=f"yt_{t}")
                    yts[t] = yt
                    for jp in range(NCH // 2):  # jc pairs share a 2-bank tile
                        ps = psAp.tile([P, 2 * HW], f32, tag="ps")
                        for h in range(2):
                            jc = 2 * jp + h
                            _band_pass(
                                nc,
                                ps[:, h * HW : (h + 1) * HW],
                                lambda kc: src[:, kc, jc * P : (jc + 1) * P],
                                bm,
                                sim_safe,
                            )
                        # psum col v holds Yt[i = (v - 3) mod 512]; un-rotate
                        # during the copy (split at col 3, same total cost) so
                        # yt col u = Yt[u]: pass-2 stationary slices start at
                        # ic*128 — 4-byte aligned, keeping LDWEIGHTS on the
                        # fast path — and need no wrap halo at all
                        jc0, jc1 = 2 * jp, 2 * jp + 1
                        ps3 = ps[:].rearrange("p (a b) -> p a b", a=2)
                        # alternate copy engines so a tensor's two pair-copies
                        # drain in parallel on ScalarE+VectorE — pass 2's first
                        # groups wait on the last of them
                        # tiny 3-col tail FIRST (it gates pass-2's ic=3 fill)
                        # and on the opposite engine of the 509-col main copy
                        on_dve = (jp + (0 if t == "x" else 1)) % 2 == 0
                        if on_dve:
                            nc.scalar.copy(
                                out=yt[:, jc0 : jc1 + 1, HW - PAD : HW],
                                in_=ps3[:, :, 0:PAD],
                            )
                            nc.vector.tensor_copy(
                                out=yt[:, jc0 : jc1 + 1, 0 : HW - PAD],
                                in_=ps3[:, :, PAD:HW],
                            )
                        else:
                            nc.vector.tensor_copy(
                                out=yt[:, jc0 : jc1 + 1, HW - PAD : HW],
                                in_=ps3[:, :, 0:PAD],
                            )
                            nc.scalar.copy(
                                out=yt[:, jc0 : jc1 + 1, 0 : HW - PAD],
                                in_=ps3[:, :, PAD:HW],
                            )

                outt = outpp.tile([P, NCH, HW], f16)
                for ip in range(NCH // 2):  # ic pairs share 2-bank tiles
                    ps1 = psZp.tile([P, 2 * HW], f32, tag="s1")
                    ps2 = psZp.tile([P, 2 * HW], f32, tag="s2")
                    # s1 halves first: the x-side yt copies (DVE) land before
                    # the x2-side (ACT), so the PE is never input-starved
                    for h in range(2):
                        ic = 2 * ip + h
                        lo = ic * P
                        _band_pass(
                            nc,
                            ps1[:, h * HW : (h + 1) * HW],
                            lambda jc: yts["x"][:, jc, lo : lo + P],
                            bm,
                            sim_safe,
                        )
                    for h in range(2):
                        ic = 2 * ip + h
                        lo = ic * P
                        _band_pass(
                            nc,
                            ps2[:, h * HW : (h + 1) * HW],
                            lambda jc: yts["x2"][:, jc, lo : lo + P],
                            bm,
                            sim_safe,
                        )
                    ts_ = tsqp.tile([P, 2 * HW], f32)
                    nc.scalar.activation(out=ts_[:], in_=ps1[:], func=SQ, scale=INV)
                    # the kernel's tail is stt -> output DMA, serial: for the
                    # final pair, split the stt per ic so the first half's DMA
                    # launches ~0.6us earlier
                    last = p == n_planes - 1 and ip == NCH // 2 - 1
                    for hs in range(2) if last else (0,):
                        sl = (
                            slice(hs * HW, (hs + 1) * HW)
                            if last
                            else slice(0, 2 * HW)
                        )
                        oc = (
                            outt[:, 2 * ip + hs, :]
                            if last
                            else outt[:, 2 * ip : 2 * ip + 2, :].rearrange(
                                "p a b -> p (a b)"
                            )
                        )
                        nc.vector.scalar_tensor_tensor(
                            out=oc,
                            in0=ps2[:, sl],
                            scalar=INV,
                            in1=ts_[:, sl],
                            op0=MUL,
                            op1=SUB,
                        )
                # output cols are rotated by +3: col c holds Var[., (c-3)%512];
                # both pieces drain per ic pair so the last plane's tail chain
                # is one stt + one DMA, not the whole plane's
                od = o_d[p].rearrange("(ic q) c -> q ic c", q=P)
                if p == n_planes - 1:
                    groups = [slice(0, 2), slice(2, 3), slice(3, 4)]
                else:
                    groups = [slice(0, 2), slice(2, 4)]
                for s in groups:
                    nc.sync.dma_start(
                        out=od[:, s, 0 : HW - PAD], in_=outt[:, s, PAD:HW]
                    )
                    nc.sync.dma_start(
                        out=od[:, s, HW - PAD : HW], in_=outt[:, s, 0:PAD]
                    )
    nc.compile()
    return nc


_CACHED = {}


def _get_nc(n_planes=PLANES_PER_CORE):
    if n_planes not in _CACHED:
        _CACHED[n_planes] = build(n_planes)
    return _CACHED[n_planes]


def kernel(x: np.ndarray) -> np.ndarray:
    from concourse.bass_utils import run_bass_kernel_spmd

    N, C, H, W = x.shape
    assert (H, W) == (HW, HW), (H, W)
    planes = np.ascontiguousarray(x.reshape(N * C, H, W).astype(np.float32))
    total = N * C
    per_core = total // N_CORES
    assert per_core == PLANES_PER_CORE, (total, N_CORES)

    bmat = _make_bmat(np.float16)
    nc = _get_nc(per_core)

    in_maps = [
        {
            "x": np.ascontiguousarray(planes[i * per_core : (i + 1) * per_core]),
            "bmat": bmat,
        }
        for i in range(N_CORES)
    ]
    res = run_bass_kernel_spmd(nc, in_maps, list(range(N_CORES)))
    out = np.concatenate([r["out"] for r in res.results], axis=0)
    return out.reshape(N, C, H, W).astype(np.float32)

